# revision 1
# baseline (speedup 1.0000x reference)
"""DGL MPNN layer on 8 Trainium2 NeuronCores.

Math (per reference):
    w_e  = (ef_e @ We + be).reshape(32, 32)          # per-edge weight
    msg_e = nf[src_e] @ w_e                          # (32,)
    out_n = sum_{e: dst_e==n} msg_e + nf_n + bias

Device pipeline per 128-edge chunk:
    x^T   <- transposing SBUF dma_gather of f16 node table (features land on
             partitions 32r+i, r = chunk%4 row strip)
    Z     <- PE matmul x @ Wcat, Wcat[i, 32d+o] = We[d, 32i+o] (+ Be block)
    P     <- DVE broadcast-mult  Z[:, :512] * ef'  (ef' re-read via 0-stride AP)
    msg   <- DVE strided reduce over d (17 terms incl. Be block), f16 out
    agg   <- PE one-hot scatter-matmul: sel_chunk^T @ msg accumulated in PSUM
             per 128-node tile (sel one-hot blocks built host-side, f16)
    out   <- nf + bias + agg, single linear DMA at the end

Sharding: edges partitioned by dst node range (6250 nodes/core); within a
core, edges sorted by local dst and padded so every 128-node tile owns
exactly CPT chunks (SPMD-uniform control flow; pad rows have all-zero sel).
"""

import numpy as np

N, E, HID, ED = 50000, 200000, 32, 16
NCORES = 8
NPC = N // NCORES            # 6250 nodes per core
NT = 49                      # node tiles per core (49*128 = 6272 >= 6250)
NPC_PAD = NT * 128
GRAN = 2048                  # edges per gather granule
CH = GRAN // 128             # chunks per granule


def _prep(nf, initial_ef, src, dst, We, be, bias):
    nf = np.ascontiguousarray(np.asarray(nf, dtype=np.float32))
    ef = np.ascontiguousarray(np.asarray(initial_ef, dtype=np.float32))
    src = np.asarray(src).astype(np.int64)
    dst = np.asarray(dst).astype(np.int64)
    We = np.asarray(We, dtype=np.float32)
    be = np.asarray(be, dtype=np.float32)
    bias = np.asarray(bias, dtype=np.float32)

    # Combined weight (32, 544): cols 32d+o for d<16, then Be at 512.
    W3 = We.reshape(ED, HID, HID)                      # [d, i, o]
    Wcat = np.empty((HID, 544), np.float32)
    for d in range(ED):
        Wcat[:, 32 * d:32 * d + 32] = W3[d]
    Wcat[:, 512:544] = be.reshape(HID, HID)
    Wcat4 = np.zeros((128, 544), np.float16)
    for r in range(4):
        Wcat4[32 * r:32 * r + 32, :] = Wcat.astype(np.float16)

    core_of = dst // NPC
    cores = []
    cpt_max = 1
    u_max = 0
    for c in range(NCORES):
        eidx = np.nonzero(core_of == c)[0]
        dl = (dst[eidx] - c * NPC).astype(np.int64)
        order = np.argsort(dl, kind="stable")
        eidx = eidx[order]
        dl = dl[order]
        tile_of = dl // 128
        counts = np.bincount(tile_of, minlength=NT)
        cpt_max = max(cpt_max, int(np.ceil(counts.max() / 128)))
        uniq = np.unique(src[eidx])
        u_max = max(u_max, len(uniq))
        srcloc = np.searchsorted(uniq, src[eidx]).astype(np.int64)
        cores.append((eidx, dl, counts, uniq, srcloc, c))

    CPT = cpt_max
    n_chunks = NT * CPT
    E_cmp = n_chunks * 128
    E_pad = ((E_cmp + GRAN - 1) // GRAN) * GRAN
    U_pad = ((u_max + 127) // 128) * 128

    in_maps = []
    for eidx, dl, counts, uniq, srcloc, c in cores:
        U = len(uniq)
        tab = np.zeros((U_pad, 128), np.float16)
        nfh = nf[uniq].astype(np.float16)
        for r in range(4):
            tab[:U, 32 * r:32 * r + 32] = nfh

        srcs = np.zeros(E_pad, np.int64)
        efs = np.zeros((E_pad, ED), np.float32)
        sel = np.zeros((E_pad, 128), np.float16)
        pos = 0            # position within the sorted edge stream
        for a in range(NT):
            n_a = int(counts[a])
            s0 = a * CPT * 128
            sl = slice(pos, pos + n_a)
            srcs[s0:s0 + n_a] = srcloc[sl]
            efs[s0:s0 + n_a] = ef[eidx[sl]]
            sel[s0 + np.arange(n_a), dl[sl] - 128 * a] = 1.0
            pos += n_a

        srcw = np.tile(srcs.astype(np.int16).reshape(E_pad // 16, 16).T, (8, 1))

        nfb = np.zeros((NPC_PAD, HID), np.float32)
        nfb[:NPC] = nf[c * NPC:(c + 1) * NPC]

        in_maps.append({
            "nft": tab,
            "wcat": Wcat4,
            "srcw": np.ascontiguousarray(srcw),
            "efs": efs,
            "seld": sel,
            "nfb": nfb,
            "bias1": bias.reshape(1, HID).copy(),
            "ones1": np.ones((1, 128), np.float32),
        })
    return in_maps, CPT, E_pad, U_pad


def build_nc(CPT, E_pad, U_pad):
    import concourse.bass as bass
    import concourse.bacc as bacc
    import concourse.mybir as mybir
    import concourse.tile as tile

    f16 = mybir.dt.float16
    f32 = mybir.dt.float32
    i16 = mybir.dt.int16
    import os
    G = E_pad // GRAN
    A = U_pad // 128
    n_chunks = NT * CPT
    kmax = int(os.environ.get("KMAX_CHUNKS", "0"))
    if kmax:
        n_chunks = min(n_chunks, kmax)
        G = min(G, (n_chunks * 128 + GRAN - 1) // GRAN)

    nc = bacc.Bacc("TRN2", target_bir_lowering=False, debug=False)
    nft = nc.dram_tensor("nft", [U_pad, 128], f16, kind="ExternalInput")
    wcat = nc.dram_tensor("wcat", [128, 544], f16, kind="ExternalInput")
    srcw = nc.dram_tensor("srcw", [128, E_pad // 16], i16, kind="ExternalInput")
    efs = nc.dram_tensor("efs", [E_pad, ED], f32, kind="ExternalInput")
    seld = nc.dram_tensor("seld", [E_pad, 128], f16, kind="ExternalInput")
    nfb = nc.dram_tensor("nfb", [NPC_PAD, HID], f32, kind="ExternalInput")
    bias1 = nc.dram_tensor("bias1", [1, HID], f32, kind="ExternalInput")
    ones1 = nc.dram_tensor("ones1", [1, 128], f32, kind="ExternalInput")
    out = nc.dram_tensor("out", [NPC_PAD, HID], f32, kind="ExternalOutput")

    with tile.TileContext(nc) as tc:
        with (
            tc.tile_pool(name="const", bufs=1) as cpool,
            tc.tile_pool(name="xt", bufs=2) as xt_pool,
            tc.tile_pool(name="efg", bufs=2) as ef_pool,
            tc.tile_pool(name="idx", bufs=2) as idx_pool,
            tc.tile_pool(name="sel", bufs=3) as sel_pool,
            tc.tile_pool(name="prod", bufs=3) as p_pool,
            tc.tile_pool(name="msg", bufs=3) as msg_pool,
            tc.tile_pool(name="za", bufs=3, space="PSUM") as za_pool,
            tc.tile_pool(name="zb", bufs=2, space="PSUM") as zb_pool,
            tc.tile_pool(name="agg", bufs=2, space="PSUM") as agg_pool,
            tc.tile_pool(name="bps", bufs=1, space="PSUM") as bps_pool,
        ):
            tab = cpool.tile([128, A, 128], f16)
            wc = cpool.tile([128, 544], f16)
            ones_sb = cpool.tile([1, 128], f32)
            bias_sb = cpool.tile([1, HID], f32)
            acc_slab = cpool.tile([128, NT, HID], f32)

            nc.sync.dma_start(tab[:], nft[:].rearrange("(a p) c -> p a c", p=128))
            nc.sync.dma_start(wc[:], wcat[:])
            nc.sync.dma_start(ones_sb[:], ones1[:])
            nc.sync.dma_start(bias_sb[:], bias1[:])
            nc.sync.dma_start(acc_slab[:],
                              nfb[:].rearrange("(a p) c -> p a c", p=128))

            bias_ps = bps_pool.tile([128, HID], f32)
            nc.tensor.matmul(bias_ps[:], ones_sb[:], bias_sb[:])
            bp = bias_ps[:]
            bias_bc = bass.AP(bp.tensor, bp.offset,
                              [bp.ap[0], [0, NT], bp.ap[1]])
            nc.vector.tensor_tensor(
                out=acc_slab[:], in0=acc_slab[:], in1=bias_bc,
                op=mybir.AluOpType.add,
            )

            # gathers run ahead over the whole padded stream
            xts = []
            for g in range(G):
                srcg = idx_pool.tile([128, GRAN // 16], i16, tag="srcg")
                s0 = g * (GRAN // 16)
                nc.sync.dma_start(srcg[:], srcw[:, s0:s0 + GRAN // 16])
                xt = xt_pool.tile([128, 1, GRAN], f16)
                nc.gpsimd.dma_gather(
                    xt[:], tab[:], srcg[:],
                    num_idxs=GRAN, num_idxs_reg=GRAN, elem_size=128,
                    transpose=True, single_packet=False,
                    sbuf_tokens_per_rank=128,
                    sbuf_free_dim_per_rank=256,
                )
                xts.append(xt)

            zbp = None
            agg = None
            for c in range(n_chunks):
                a, k = c // CPT, c % CPT
                r = c % 4
                g, ci = c // CH, c % CH
                if r == 0:
                    zbp = zb_pool.tile([128, 4, HID], f32)
                if k == 0:
                    agg = agg_pool.tile([128, HID], f32)

                efg = ef_pool.tile([128, ED], f32)
                nc.sync.dma_start(
                    efg[:],
                    efs[c * 128:(c + 1) * 128, :])
                sel_t = sel_pool.tile([128, 128], f16)
                nc.sync.dma_start(sel_t[:], seld[c * 128:(c + 1) * 128, :])

                za = za_pool.tile([128, 512], f32)
                lhsT = xts[g][32 * r:32 * r + 32, 0, 128 * ci:128 * ci + 128]
                nc.tensor.matmul(za[:], lhsT, wc[32 * r:32 * r + 32, 0:512],
                                 tile_position=(32 * r, 0))
                nc.tensor.matmul(zbp[:, r, :], lhsT,
                                 wc[32 * r:32 * r + 32, 512:544],
                                 tile_position=(32 * r, 0))

                P = p_pool.tile([128, 544], f32)
                ea = efg[:]
                ef_bc = bass.AP(ea.tensor, ea.offset,
                                [ea.ap[0], ea.ap[1], [0, HID]])
                nc.vector.tensor_tensor(
                    out=P[:, 0:512].rearrange("p (d o) -> p d o", o=HID),
                    in0=za[:].rearrange("p (d o) -> p d o", o=HID),
                    in1=ef_bc,
                    op=mybir.AluOpType.mult,
                )
                nc.scalar.copy(P[:, 512:544], zbp[:, r, :])
                msg_t = msg_pool.tile([128, HID], f16)
                with nc.allow_low_precision("accumulated in f32 by PSUM next"):
                    nc.vector.tensor_reduce(
                        out=msg_t[:],
                        in_=P[:].rearrange("p (d o) -> p o d", o=HID),
                        axis=mybir.AxisListType.X,
                        op=mybir.AluOpType.add,
                    )
                nc.tensor.matmul(agg[:], sel_t[:], msg_t[:],
                                 start=(k == 0), stop=(k == CPT - 1))
                if k == CPT - 1:
                    nc.vector.tensor_tensor(
                        out=acc_slab[:, a, :], in0=acc_slab[:, a, :],
                        in1=agg[:], op=mybir.AluOpType.add)

            nc.sync.dma_start(out[:].rearrange("(a p) c -> p a c", p=128),
                              acc_slab[:])
    nc.compile()
    return nc


_CACHE = {}


def kernel(nf, initial_ef, src, dst, We, be, bias):
    in_maps, CPT, E_pad, U_pad = _prep(nf, initial_ef, src, dst, We, be, bias)
    key = (CPT, E_pad, U_pad)
    if key not in _CACHE:
        _CACHE[key] = build_nc(CPT, E_pad, U_pad)
    nc = _CACHE[key]

    from concourse.bass_utils import run_bass_kernel_spmd
    res = run_bass_kernel_spmd(nc, in_maps, core_ids=list(range(NCORES)))
    outs = [r["out"][:NPC, :HID] for r in res.results]
    return np.ascontiguousarray(np.concatenate(outs, axis=0).astype(np.float32))

